# revision 1
# baseline (speedup 1.0000x reference)
"""DCT-feature-extractor kernel for 8 Trainium2 NeuronCores.

Math collapse: the reference keeps only dct[0, 0:4] of each 8x8 block's 2-D
orthonormal-DFT real part.  Row 0 of the DFT matrix is constant (Fr[0,:] =
1/sqrt(8), Fi[0,:] = 0), so

    feat[m] = sum_l G[m, l] * colsum[l],   G[m, l] = cos(2*pi*m*l/8) / 8,

where colsum[l] is the column sum of the 8x8 block.  The whole module is then

    out[b, o] = sum_{i,j,m} W[o, (i*64+j)*4+m] * feat[b,i,j,m] + bias[o].

Sharding: split the 512 image rows (block-row groups i) and the matching
weight columns across 8 cores -> each core reads its image slice + weight
shard (no replication) and emits a [32, 512] partial product; the host sums
partials and adds the bias.

The rel-err budget (2e-2) is far above fp16 rounding (~5e-4), so the host
casts both streams to float16: per-core HBM traffic drops from 8.6 MB to
4.3 MB, which halves the BW-bound stream phase.

Layout trick: x is host-shuffled to partitions = (j16, l8) (the w-column
within a 128-col group) and free = (a8, i8, b32).  The column sum then runs
along the free dim (three stride-1 fp16 DVE adds at the 2x 16-bit rate) and
lands ALREADY TRANSPOSED: ys[(j,l), (i,b)].  feats follows as one plain
matmul with the block-diagonal G — no PE transposes, no identity const, no
yT copies.

Per-core schedule (HWDGE transfers start in program order on the SP ring):
  SP ring:  Gblk consts (16 KB), x in 4 chunks (one per 128-col w-group),
            then the weight in 4 chunks of 4 output-tiles each; the final
            matmuls chase the arriving weight chunks.
  DVE: column-sum tree per x chunk, then the 2-group PSUM collapse
       (copy + add) at the end.
  ACT: featsT PSUM->SBUF (fp32->fp16) copies.
  PE:  4 G matmuls + 16 accumulating stage-3 matmuls over 2 PE column
       groups (adjacent t alternate groups so LDWEIGHTS overlaps).
  Sync issues the final out DMA on its (warm) ring.
The Bass entry barrier is stripped (it only guards unused framework const
memsets) so DMA descriptors issue as soon as the runtime prologue ends.
"""

import numpy as np

import concourse.bacc as bacc
import concourse.mybir as mybir
from concourse.bass_utils import run_bass_kernel_spmd
from concourse.tile import TileContext

N_CORES = 8
B = 32            # batch
H = 512           # image height
WD = 512          # image width
BS = 8            # dct block size
NF = 4            # kept dct coefficients per block
OUT = 512         # linear output dim
RPC = H // N_CORES          # 64 rows per core
IPC = RPC // BS             # 8 block-rows per core
F32 = mybir.dt.float32
F16 = mybir.dt.float16

NCONST = 64       # Gblk columns
NT = 2 * IPC      # 16 stage-3 matmuls (q-chunks of 128)
XCK = BS * IPC * B          # 2048 free cols per x chunk (a8, i8, b32)


def _g_mat():
    m = np.arange(NF)[:, None].astype(np.float64)
    l = np.arange(BS)[None, :].astype(np.float64)
    return (np.cos(2.0 * np.pi * m * l / BS) / 8.0).astype(np.float32)  # [4, 8]


def _consts():
    """Gblk [p=(j16,l8), q=(j16,m4)] = G[m,l] * (j16 == j16')."""
    g = _g_mat()
    block = np.zeros((128, 64), np.float32)
    for j in range(16):
        block[j * 8:(j + 1) * 8, j * 4:(j + 1) * 4] = g.T  # [l, m]
    return block.astype(np.float16)


def _build_bass():
    nc = bacc.Bacc("TRN2", target_bir_lowering=False, debug=False)
    # Strip the Bass.__init__ entry barrier (drain + event-sem per engine):
    # it only guards framework const-AP memsets this kernel never reads, and
    # it stalls the DMA queues behind the slow-to-start Tensor engine.
    entry = nc.main_func.blocks[0]
    for inst in [
        i for i in entry.instructions
        if isinstance(i, (mybir.InstDrain, mybir.InstEventSemaphore, mybir.InstMemset))
    ]:
        entry.instructions.remove(inst)
    # x host-prepped: [wgp2, p=(j16,l8), f=(wg2, a8, i8, b32)]  (fp16)
    x = nc.dram_tensor("x", [2, 128, 2 * XCK], F16, kind="ExternalInput")
    # wt host-prepped: [p, 64 Gblk | t=(p2,i8) x o]  (fp16)
    wt = nc.dram_tensor("wt", [128, NCONST + NT * OUT], F16, kind="ExternalInput")
    # both PSUM column-group partials go out on an fp16 wire; the host
    # upcasts and sums them (it already sums the 8 per-core partials, so 16
    # summands cost nothing extra, and partials are O(0.3) so fp16 rounding
    # adds ~5e-4 relative error against a 2e-2 budget)
    out = nc.dram_tensor("out", [2 * B, OUT], F16, kind="ExternalOutput")

    with TileContext(nc) as tc:
        with (
            tc.tile_pool(name="sb", bufs=1) as sb,
            tc.tile_pool(name="ps", bufs=1, space="PSUM") as ps,
        ):
            # ---- DMA program order == HWDGE FIFO order on the SP ring ----
            # Gblk goes on the gpsimd SWDGE ring so x leads the SP ring;
            # x moves as 2x 1MB chunks (8KB-contiguous rows), the weight as
            # 1MB + 512KB + 2x 256KB so only one matmul pair trails the
            # stream end.
            wts = sb.tile([128, NCONST + NT * OUT], F16, tag="wt")
            xs = [
                sb.tile([128, 2 * XCK], F16, tag=f"x{c}", name=f"x{c}")
                for c in range(2)
            ]
            for c in range(2):
                nc.sync.dma_start(out=xs[c][:, :], in_=x.ap()[c])
            nc.gpsimd.dma_start(out=wts[:, 0:NCONST], in_=wt.ap()[:, 0:NCONST])
            gblk = wts[:, 0:NCONST]
            wbnd = [0, 4, 8, 12, 14, 16]   # weight chunks in t-tiles
            for k in range(len(wbnd) - 1):
                lo, hi = NCONST + wbnd[k] * OUT, NCONST + wbnd[k + 1] * OUT
                nc.sync.dma_start(out=wts[:, lo:hi], in_=wt.ap()[:, lo:hi])

            # ---- stage 1: partial column sums along the free dim (DVE) ----
            # Two tree levels only: ys[c][(j,l), (a2,i,b)] = partial sums
            # with a-parity kept; the G matmul contracts the parity away
            # (two accumulating rhs halves), shortening the DVE chain.
            ys = [sb.tile([128, 512], F16, tag=f"y{c}", name=f"y{c}") for c in range(4)]

            def colsum(c):
                t, b0 = xs[c // 2], (c % 2) * XCK
                nc.vector.tensor_add(
                    t[:, b0:b0 + 1024], t[:, b0:b0 + 1024], t[:, b0 + 1024:b0 + 2048])
                nc.vector.tensor_add(
                    ys[c][:, :], t[:, b0:b0 + 512], t[:, b0 + 512:b0 + 1024])

            # ---- stage 2: featsT = Gblk^T @ ys, one PSUM tile per w-group
            # (separate tiles so the ACT copy of one w-group doesn't
            # false-serialize the next w-group's matmul)
            pft = [ps.tile([128, 256], F32, tag=f"pft{c}", name=f"pft{c}") for c in range(4)]
            ftp = [sb.tile([128, 256], F16, tag=f"ft{p}", name=f"ft{p}") for p in range(2)]

            def feats(c):               # w-group c -> half of pair p
                p, wg2 = divmod(c, 2)
                for a2 in range(2):
                    nc.tensor.matmul(
                        pft[c][64 * wg2:64 * (wg2 + 1), :],
                        gblk, ys[c][:, 256 * a2:256 * (a2 + 1)],
                        start=(a2 == 0), stop=(a2 == 1),
                        tile_position=(0, 64 * wg2),
                        skip_group_check=True,
                    )
                nc.scalar.copy(
                    ftp[p][64 * wg2:64 * (wg2 + 1), :],
                    pft[c][64 * wg2:64 * (wg2 + 1), :],
                )

            # ---- stage 3: 16 accumulating matmuls over 2 PE column groups
            pout = ps.tile([128, OUT], F32, tag="pout")

            def stage3(t):
                p, i = divmod(t, IPC)
                g = t % 2
                nc.tensor.matmul(
                    pout[32 * g:32 * (g + 1), :],
                    ftp[p][:, i * 32:(i + 1) * 32],
                    wts[:, NCONST + t * OUT:NCONST + (t + 1) * OUT],
                    start=(t < 2),
                    stop=(t >= NT - 2),
                    tile_position=(0, 32 * g),
                    skip_group_check=True,
                )

            colsum(0)
            colsum(1)
            feats(0)
            feats(1)
            for t in range(0, 8):
                stage3(t)
            colsum(2)
            colsum(3)
            feats(2)
            feats(3)
            for t in range(8, NT):
                stage3(t)

            # ---- ship both column-group partials (no on-device add) ----
            # copy halves split across DVE and ACT so they run in parallel
            outs = sb.tile([2 * B, OUT], F16, tag="outs")
            nc.vector.tensor_copy(outs[:, 0:OUT // 2], pout[0:2 * B, 0:OUT // 2])
            nc.scalar.copy(outs[:, OUT // 2:OUT], pout[0:2 * B, OUT // 2:OUT])
            nc.sync.dma_start(out=out.ap(), in_=outs[:, :])

    nc.compile()
    return nc


_NC_CACHE = None


def _get_nc():
    global _NC_CACHE
    if _NC_CACHE is None:
        _NC_CACHE = _build_bass()
    return _NC_CACHE


_CST = _consts()


def make_in_maps(imgs, weight):
    """Per-core input dicts: shuffled channel-0 row slice + weight shard."""
    wr = weight.reshape(OUT, H // BS, WD // BS, NF)  # [o, i_glob, j, m]
    in_maps = []
    for c in range(N_CORES):
        xc = imgs[:, 0, RPC * c:RPC * (c + 1), :]    # [32, 64, 512]
        # [b, (i,a), (wgp, wg2, j16, l)] -> [wgp, (j16, l), (wg2, a, i, b)]
        xd = xc.reshape(B, IPC, BS, 2, 2, 128).transpose(3, 5, 4, 2, 1, 0)
        xd = np.ascontiguousarray(xd.reshape(2, 128, 2 * XCK).astype(np.float16))
        wc = wr[:, IPC * c:IPC * (c + 1)]            # [o, i, j, m]
        # q = wg2*64 + j16*4 + m  (j = (2p + wg2)*16 + j16),  t = p*8 + i
        wtc = wc.reshape(OUT, IPC, 2, 2, 16, NF)     # o, i, p, wg2, j16, m
        wtc = wtc.transpose(3, 4, 5, 2, 1, 0)        # wg2, j16, m, p, i, o
        wtc = np.concatenate(
            [_CST, wtc.reshape(128, NT * OUT).astype(np.float16)], axis=1)
        in_maps.append({"x": xd, "wt": np.ascontiguousarray(wtc)})
    return in_maps


def kernel(imgs_tensors, weight, bias, block_size=8, num_features=4, **_):
    assert int(block_size) == BS and int(num_features) == NF
    imgs = np.ascontiguousarray(np.asarray(imgs_tensors, dtype=np.float32))
    w = np.ascontiguousarray(np.asarray(weight, dtype=np.float32))
    b = np.asarray(bias, dtype=np.float32)
    assert imgs.shape == (B, 3, H, WD) and w.shape == (OUT, H // BS * WD // BS * NF)

    nc = _get_nc()
    res = run_bass_kernel_spmd(nc, make_in_maps(imgs, w), core_ids=list(range(N_CORES)))
    acc = np.zeros((B, OUT), np.float32)
    for r in res.results:
        po = r["out"].astype(np.float32)
        acc += po[0:B] + po[B:2 * B]
    return (acc + b[None, :]).astype(np.float32)



# revision 3
# speedup vs baseline: 1.0612x; 1.0612x over previous
"""DCT-feature-extractor kernel for 8 Trainium2 NeuronCores.

Math collapse: the reference keeps only dct[0, 0:4] of each 8x8 block's 2-D
orthonormal-DFT real part.  Row 0 of the DFT matrix is constant (Fr[0,:] =
1/sqrt(8), Fi[0,:] = 0), so

    feat[m] = sum_l G[m, l] * colsum[l],   G[m, l] = cos(2*pi*m*l/8) / 8,

where colsum[l] is the column sum of the 8x8 block.  The whole module is then

    out[b, o] = sum_{i,j,m} W[o, (i*64+j)*4+m] * feat[b,i,j,m] + bias[o].

Sharding: split the 512 image rows (block-row groups i) and the matching
weight columns across 8 cores -> each core reads its image slice + weight
shard (no replication) and emits per-PE-group partial products; the host sums
partials, rescales, and adds the bias.

Precision: x streams as fp16 (rel err ~5e-4).  The weight streams as fp8
e3m4 scaled by 256 (host-measured end-to-end rel err 1.31e-2 against the
2e-2 budget) which halves the weight bytes: per-core HBM traffic drops from
4.3 MB (all-fp16) to ~3.3 MB.  The stage-3 matmuls run mixed fp16 lhsT x
fp8e3 rhs; the host divides the partials by 256.

Layout trick: x is host-shuffled to partitions = (j16, l8) (the w-column
within a 128-col group) and free = (a8, i8, b32), one chunk per w-group.
The column sum then runs along the free dim (stride-1 fp16 DVE adds) and
lands ALREADY TRANSPOSED: ys[(j,l), (a2,i,b)]; the a-parity is contracted
away by the accumulating G matmul.  feats follows as one plain matmul with
the block-diagonal G -- no PE transposes, no identity const, no yT copies.

Per-core schedule (HWDGE transfers start in program order per ring):
  SP ring:     x in 4 chunks (one per w-group, 4KB rows), then the weight in
               5 chunks (4+4+4+2+2 output-tiles); the final out DMA.
  ACT ring:    the small Gblk const (16 KB) so it beats the x stream without
               delaying it.
  DVE:  column-sum tree per x chunk, then half the pout->outs cast.
  ACT:  featsT PSUM->SBUF (fp32->fp16) copies + other half of the out cast.
  PE:   8 G matmuls + 16 accumulating stage-3 matmuls spread over FOUR
        32-wide PE column groups (t mod 4) so up to 4 matmuls stream
        concurrently through different subarray columns; the host sums the
        4 PSUM row-blocks.
The Bass entry barrier is stripped (it only guards unused framework const
memsets), and the second exit butterfly barrier is stripped (the first
barrier + sem-range clear already quiesce everything the next execution
needs).
"""

import numpy as np
import ml_dtypes

import concourse.bacc as bacc
import concourse.mybir as mybir
from concourse.bass_utils import run_bass_kernel_spmd
from concourse.tile import TileContext

N_CORES = 8
B = 32            # batch
H = 512           # image height
WD = 512          # image width
BS = 8            # dct block size
NF = 4            # kept dct coefficients per block
OUT = 512         # linear output dim
RPC = H // N_CORES          # 64 rows per core
IPC = RPC // BS             # 8 block-rows per core
F32 = mybir.dt.float32
F16 = mybir.dt.float16
F8 = mybir.dt.float8e3     # e3m4: 4 mantissa bits
WSCALE = 256.0             # host multiplies W, divides partials

NCONST = 64       # Gblk columns
NT = 2 * IPC      # 16 stage-3 matmuls (q-chunks of 128)
XCK = BS * IPC * B // 4 * 4  # free cols per w-group x chunk: (a8, i8, b32)
XCK = BS * IPC * B           # 2048
NG = 4            # stage-3 PE column groups (t mod 4)


def _g_mat():
    m = np.arange(NF)[:, None].astype(np.float64)
    l = np.arange(BS)[None, :].astype(np.float64)
    return (np.cos(2.0 * np.pi * m * l / BS) / 8.0).astype(np.float32)  # [4, 8]


def _consts():
    """Gblk [p=(j16,l8), q=(j16,m4)] = G[m,l] * (j16 == j16')."""
    g = _g_mat()
    block = np.zeros((128, 64), np.float32)
    for j in range(16):
        block[j * 8:(j + 1) * 8, j * 4:(j + 1) * 4] = g.T  # [l, m]
    return block.astype(np.float16)


def _build_bass():
    nc = bacc.Bacc("TRN2", target_bir_lowering=False, debug=False)
    # Strip the Bass.__init__ entry barrier (drain + event-sem per engine):
    # it only guards framework const-AP memsets this kernel never reads, and
    # it stalls the DMA queues behind the slow-to-start Tensor engine.
    entry = nc.main_func.blocks[0]
    for inst in [
        i for i in entry.instructions
        if isinstance(i, (mybir.InstDrain, mybir.InstEventSemaphore, mybir.InstMemset))
    ]:
        entry.instructions.remove(inst)
    # x host-prepped: [wg4, p=(j16,l8), f=(a8, i8, b32)]  (fp16)
    x = nc.dram_tensor("x", [4, 128, XCK], F16, kind="ExternalInput")
    # Gblk const (fp16), on the ACT ring
    gb = nc.dram_tensor("gb", [128, NCONST], F16, kind="ExternalInput")
    # wt host-prepped: [p, t=(p2,i8) x o]  (fp8 e3m4, x256)
    wt = nc.dram_tensor("wt", [128, NT * OUT], F8, kind="ExternalInput")
    # all four PSUM column-group partials ship out as fp16; the host upcasts,
    # sums the 4 groups x 8 cores, divides by WSCALE and adds the bias.
    out = nc.dram_tensor("out", [NG * B, OUT], F16, kind="ExternalOutput")

    with TileContext(nc) as tc:
        with (
            tc.tile_pool(name="sb", bufs=1) as sb,
            tc.tile_pool(name="ps", bufs=1, space="PSUM") as ps,
        ):
            # ---- DMA program order == HWDGE FIFO order per ring ----
            gbs = sb.tile([128, NCONST], F16, tag="gb")
            wts = sb.tile([128, NT * OUT], F8, tag="wt")
            xs = [
                sb.tile([128, XCK], F16, tag=f"x{c}", name=f"x{c}")
                for c in range(4)
            ]
            for c in range(4):
                nc.sync.dma_start(out=xs[c][:, :], in_=x.ap()[c])
            nc.scalar.dma_start(out=gbs[:, :], in_=gb.ap())
            wbnd = [0, 4, 8, 12, 14, 16]   # weight chunks in t-tiles
            for k in range(len(wbnd) - 1):
                lo, hi = wbnd[k] * OUT, wbnd[k + 1] * OUT
                nc.sync.dma_start(out=wts[:, lo:hi], in_=wt.ap()[:, lo:hi])

            # ---- stage 1: partial column sums along the free dim (DVE) ----
            # Two tree levels only: ys[c][(j,l), (a2,i,b)] = partial sums
            # with a-parity kept; the G matmul contracts the parity away
            # (two accumulating rhs halves), shortening the DVE chain.
            ys = [sb.tile([128, 512], F16, tag=f"y{c}", name=f"y{c}") for c in range(4)]

            def colsum(c):
                t = xs[c]
                nc.vector.tensor_add(
                    t[:, 0:1024], t[:, 0:1024], t[:, 1024:2048])
                nc.vector.tensor_add(
                    ys[c][:, :], t[:, 0:512], t[:, 512:1024])

            # ---- stage 2: featsT = Gblk^T @ ys, one PSUM tile per w-group
            # (separate tiles so the ACT copy of one w-group doesn't
            # false-serialize the next w-group's matmul)
            pft = [ps.tile([128, 256], F32, tag=f"pft{c}", name=f"pft{c}") for c in range(4)]
            ftp = [sb.tile([128, 256], F16, tag=f"ft{p}", name=f"ft{p}") for p in range(2)]

            def feats(c):               # w-group c -> half of pair p
                p, wg2 = divmod(c, 2)
                for a2 in range(2):
                    nc.tensor.matmul(
                        pft[c][64 * wg2:64 * (wg2 + 1), :],
                        gbs[:, :], ys[c][:, 256 * a2:256 * (a2 + 1)],
                        start=(a2 == 0), stop=(a2 == 1),
                        tile_position=(0, 64 * wg2),
                        skip_group_check=True,
                    )
                nc.scalar.copy(
                    ftp[p][64 * wg2:64 * (wg2 + 1), :],
                    pft[c][64 * wg2:64 * (wg2 + 1), :],
                )

            # ---- stage 3: 16 accumulating matmuls over 4 PE column groups
            pout = ps.tile([128, OUT], F32, tag="pout")

            def stage3(t):
                p, i = divmod(t, IPC)
                g = t % NG
                nc.tensor.matmul(
                    pout[32 * g:32 * (g + 1), :],
                    ftp[p][:, i * 32:(i + 1) * 32],
                    wts[:, t * OUT:(t + 1) * OUT],
                    start=(t < NG),
                    stop=(t >= NT - NG),
                    tile_position=(0, 32 * g),
                    skip_group_check=True,
                )

            colsum(0)
            colsum(1)
            feats(0)
            feats(1)
            for t in range(0, 6):
                stage3(t)
            colsum(2)
            colsum(3)
            feats(2)
            feats(3)
            for t in range(6, NT):
                stage3(t)

            # ---- ship all four column-group partials (no on-device add) ----
            # copy halves split across DVE and ACT so they run in parallel
            outs = sb.tile([NG * B, OUT], F16, tag="outs")
            nc.vector.tensor_copy(outs[:, 0:OUT // 2], pout[0:NG * B, 0:OUT // 2])
            nc.scalar.copy(outs[:, OUT // 2:OUT], pout[0:NG * B, OUT // 2:OUT])
            nc.sync.dma_start(out=out.ap(), in_=outs[:, :])

    # Strip the second exit butterfly barrier: the exit block ends with
    # [tile-drain(SP, sem waits), butterfly A, InstISA sem-range-clear,
    # butterfly B].  Butterfly A + the clear already quiesce every engine and
    # reset the tile sems for the next execution of this NEFF; butterfly B
    # only delays the runtime epilogue.
    exit_blk = nc.main_func.blocks[-1]
    insts = exit_blk.instructions
    isa_idx = max(i for i, ins in enumerate(insts) if isinstance(ins, mybir.InstISA))
    tail = insts[isa_idx + 1:]
    assert all(isinstance(i, (mybir.InstDrain, mybir.InstEventSemaphore)) for i in tail), \
        [type(i).__name__ for i in tail]
    del insts[isa_idx + 1:]

    nc.compile()
    return nc


_NC_CACHE = None


def _get_nc():
    global _NC_CACHE
    if _NC_CACHE is None:
        _NC_CACHE = _build_bass()
    return _NC_CACHE


_CST = _consts()


def make_in_maps(imgs, weight):
    """Per-core input dicts: shuffled channel-0 row slice + weight shard."""
    wr = weight.reshape(OUT, H // BS, WD // BS, NF)  # [o, i_glob, j, m]
    in_maps = []
    for c in range(N_CORES):
        xc = imgs[:, 0, RPC * c:RPC * (c + 1), :]    # [32, 64, 512]
        # [b, (i,a), (wg4, j16, l)] -> [wg4, (j16, l), (a, i, b)]
        xd = xc.reshape(B, IPC, BS, 4, 128).transpose(3, 4, 2, 1, 0)
        xd = np.ascontiguousarray(xd.reshape(4, 128, XCK).astype(np.float16))
        wc = wr[:, IPC * c:IPC * (c + 1)]            # [o, i, j, m]
        # q = wg2*64 + j16*4 + m  (j = (2p + wg2)*16 + j16),  t = p*8 + i
        wtc = wc.reshape(OUT, IPC, 2, 2, 16, NF)     # o, i, p, wg2, j16, m
        wtc = wtc.transpose(3, 4, 5, 2, 1, 0)        # wg2, j16, m, p, i, o
        wtc = (wtc.reshape(128, NT * OUT) * WSCALE).astype(ml_dtypes.float8_e3m4)
        in_maps.append({
            "x": xd,
            "gb": np.ascontiguousarray(_CST),
            "wt": np.ascontiguousarray(wtc),
        })
    return in_maps


def kernel(imgs_tensors, weight, bias, block_size=8, num_features=4, **_):
    assert int(block_size) == BS and int(num_features) == NF
    imgs = np.ascontiguousarray(np.asarray(imgs_tensors, dtype=np.float32))
    w = np.ascontiguousarray(np.asarray(weight, dtype=np.float32))
    b = np.asarray(bias, dtype=np.float32)
    assert imgs.shape == (B, 3, H, WD) and w.shape == (OUT, H // BS * WD // BS * NF)

    nc = _get_nc()
    res = run_bass_kernel_spmd(nc, make_in_maps(imgs, w), core_ids=list(range(N_CORES)))
    acc = np.zeros((B, OUT), np.float32)
    for r in res.results:
        po = r["out"].astype(np.float32)
        for g in range(NG):
            acc += po[g * B:(g + 1) * B]
    return (acc / WSCALE + b[None, :]).astype(np.float32)


# revision 6
# speedup vs baseline: 1.0869x; 1.0242x over previous
"""DCT-feature-extractor kernel for 8 Trainium2 NeuronCores.

Math collapse: the reference keeps only dct[0, 0:4] of each 8x8 block's 2-D
orthonormal-DFT real part.  Row 0 of the DFT matrix is constant (Fr[0,:] =
1/sqrt(8), Fi[0,:] = 0), so

    feat[m] = sum_l G[m, l] * colsum[l],   G[m, l] = cos(2*pi*m*l/8) / 8,

where colsum[l] is the column sum of the 8x8 block.  The whole module is then

    out[b, o] = sum_{i,j,m} W[o, (i*64+j)*4+m] * feat[b,i,j,m] + bias[o].

Sharding: split the 512 image rows (block-row groups i) and the matching
weight columns across 8 cores -> each core reads its image slice + weight
shard (no replication) and emits per-PE-group partial products; the host sums
partials, rescales, and adds the bias.

Precision: x streams as fp16 (rel err ~5e-4).  The weight streams as fp8
e3m4 scaled by 256 (host-measured end-to-end rel err 1.31e-2 against the
2e-2 budget) which halves the weight bytes: per-core HBM traffic drops from
4.3 MB (all-fp16) to ~3.3 MB.  The stage-3 matmuls run mixed fp16 lhsT x
fp8e3 rhs; the host divides the partials by 256.

Layout trick: x is host-shuffled to partitions = (j16, l8) (the w-column
within a 128-col group) and free = (a8, i8, b32), one chunk per w-group.
The column sum then runs along the free dim (stride-1 fp16 DVE adds) and
lands ALREADY TRANSPOSED: ys[(j,l), (a2,i,b)]; the a-parity is contracted
away by the accumulating G matmul.  feats follows as one plain matmul with
the block-diagonal G -- no PE transposes, no identity const, no yT copies.

Per-core schedule (HWDGE transfers start in program order per ring):
  SP ring:     x in 4 chunks (one per w-group, 4KB rows), then the weight in
               5 chunks (4+4+4+2+2 output-tiles); the final out DMA.
  ACT ring:    the small Gblk const (16 KB) so it beats the x stream without
               delaying it.
  DVE:  column-sum tree per x chunk, then half the pout->outs cast.
  ACT:  featsT PSUM->SBUF (fp32->fp16) copies + other half of the out cast.
  PE:   8 G matmuls + 16 accumulating stage-3 matmuls spread over FOUR
        32-wide PE column groups (t mod 4) so up to 4 matmuls stream
        concurrently through different subarray columns; the host sums the
        4 PSUM row-blocks.
The Bass entry barrier is stripped (it only guards unused framework const
memsets), and the second exit butterfly barrier is stripped (the first
barrier + sem-range clear already quiesce everything the next execution
needs).
"""

import numpy as np
import ml_dtypes

import concourse.bacc as bacc
import concourse.mybir as mybir
from concourse.bass_utils import run_bass_kernel_spmd
from concourse.tile import TileContext

N_CORES = 8
B = 32            # batch
H = 512           # image height
WD = 512          # image width
BS = 8            # dct block size
NF = 4            # kept dct coefficients per block
OUT = 512         # linear output dim
RPC = H // N_CORES          # 64 rows per core
IPC = RPC // BS             # 8 block-rows per core
F32 = mybir.dt.float32
F16 = mybir.dt.float16
F8 = mybir.dt.float8e3     # e3m4: 4 mantissa bits
WSCALE = 256.0             # host multiplies W, divides partials

NCONST = 64       # Gblk columns
NT = 2 * IPC      # 16 stage-3 matmuls (q-chunks of 128)
XCK = BS * IPC * B // 4 * 4  # free cols per w-group x chunk: (a8, i8, b32)
XCK = BS * IPC * B           # 2048
NG = 4            # stage-3 PE column groups (t mod 4)


def _g_mat():
    m = np.arange(NF)[:, None].astype(np.float64)
    l = np.arange(BS)[None, :].astype(np.float64)
    return (np.cos(2.0 * np.pi * m * l / BS) / 8.0).astype(np.float32)  # [4, 8]


def _consts():
    """Gblk [p=(j16,l8), q=(j16,m4)] = G[m,l] * (j16 == j16')."""
    g = _g_mat()
    block = np.zeros((128, 64), np.float32)
    for j in range(16):
        block[j * 8:(j + 1) * 8, j * 4:(j + 1) * 4] = g.T  # [l, m]
    return block.astype(np.float16)


def _build_bass():
    nc = bacc.Bacc("TRN2", target_bir_lowering=False, debug=False)
    # Strip the Bass.__init__ entry barrier (drain + event-sem per engine):
    # it only guards framework const-AP memsets this kernel never reads, and
    # it stalls the DMA queues behind the slow-to-start Tensor engine.
    entry = nc.main_func.blocks[0]
    for inst in [
        i for i in entry.instructions
        if isinstance(i, (mybir.InstDrain, mybir.InstEventSemaphore, mybir.InstMemset))
    ]:
        entry.instructions.remove(inst)
    # x host-prepped: [wgp2, p=(j16,l8), f=(wg2, a8, i8, b32)]  (fp16)
    # 2 chunks with 8KB-contiguous rows: 4KB descriptors measurably starve
    # the descriptor generator (24% of packets stall, ~280 GB/s vs 400).
    x = nc.dram_tensor("x", [2, 128, 2 * XCK], F16, kind="ExternalInput")
    # Gblk const (fp16), on the ACT ring
    gb = nc.dram_tensor("gb", [128, NCONST], F16, kind="ExternalInput")
    # wt host-prepped: [p, t=(p2,i8) x o]  (fp8 e3m4, x256)
    wt = nc.dram_tensor("wt", [128, NT * OUT], F8, kind="ExternalInput")
    # all four PSUM column-group partials ship out as fp16; the host upcasts,
    # sums the 4 groups x 8 cores, divides by WSCALE and adds the bias.
    out = nc.dram_tensor("out", [NG * B, OUT], F16, kind="ExternalOutput")

    with TileContext(nc) as tc:
        with (
            tc.tile_pool(name="sb", bufs=1) as sb,
            tc.tile_pool(name="ps", bufs=1, space="PSUM") as ps,
        ):
            # ---- DMA program order == HWDGE FIFO order per ring ----
            gbs = sb.tile([128, NCONST], F16, tag="gb")
            wts = sb.tile([128, NT * OUT], F8, tag="wt")
            xs = [
                sb.tile([128, 2 * XCK], F16, tag=f"x{c}", name=f"x{c}")
                for c in range(2)
            ]
            for c in range(2):
                nc.sync.dma_start(out=xs[c][:, :], in_=x.ap()[c])
            nc.scalar.dma_start(out=gbs[:, :], in_=gb.ap())
            wbnd = [0, 4, 8, 12, 14, 16]   # weight chunks in t-tiles
            for k in range(len(wbnd) - 1):
                lo, hi = wbnd[k] * OUT, wbnd[k + 1] * OUT
                nc.sync.dma_start(out=wts[:, lo:hi], in_=wt.ap()[:, lo:hi])

            # ---- stage 1: one partial column-sum level along the free dim
            # (DVE): ys[c][(j,l), (a4,i,b)] keeps FOUR a-phases; the G matmul
            # contracts them with 4 accumulating rhs quarters.  One DVE level
            # instead of two shortens the x1 -> ftp[1] critical chain.
            ys = [sb.tile([128, 1024], F16, tag=f"y{c}", name=f"y{c}") for c in range(4)]

            def colsum(c):
                t, b0 = xs[c // 2], (c % 2) * XCK
                nc.vector.tensor_add(
                    ys[c][:, :], t[:, b0:b0 + 1024], t[:, b0 + 1024:b0 + 2048])

            # ---- stage 2: featsT = Gblk^T @ ys, one PSUM tile per w-group
            # (separate tiles so the ACT copy of one w-group doesn't
            # false-serialize the next w-group's matmul)
            pft = [ps.tile([128, 256], F32, tag=f"pft{c}", name=f"pft{c}") for c in range(4)]
            ftp = [sb.tile([128, 256], F16, tag=f"ft{p}", name=f"ft{p}") for p in range(2)]

            def feats(c):               # w-group c -> half of pair p
                p, wg2 = divmod(c, 2)
                for a4 in range(4):
                    nc.tensor.matmul(
                        pft[c][64 * wg2:64 * (wg2 + 1), :],
                        gbs[:, :], ys[c][:, 256 * a4:256 * (a4 + 1)],
                        start=(a4 == 0), stop=(a4 == 3),
                        tile_position=(0, 64 * wg2),
                        skip_group_check=True,
                    )
                nc.scalar.copy(
                    ftp[p][64 * wg2:64 * (wg2 + 1), :],
                    pft[c][64 * wg2:64 * (wg2 + 1), :],
                )

            # ---- stage 3: 16 accumulating matmuls over 4 PE column groups
            pout = ps.tile([128, OUT], F32, tag="pout")

            def stage3(t):
                p, i = divmod(t, IPC)
                g = t % NG
                nc.tensor.matmul(
                    pout[32 * g:32 * (g + 1), :],
                    ftp[p][:, i * 32:(i + 1) * 32],
                    wts[:, t * OUT:(t + 1) * OUT],
                    start=(t < NG),
                    stop=(t >= NT - NG),
                    tile_position=(0, 32 * g),
                    skip_group_check=True,
                )

            colsum(0)
            colsum(1)
            feats(0)
            feats(1)
            for t in range(0, 6):
                stage3(t)
            colsum(2)
            colsum(3)
            feats(2)
            feats(3)
            for t in range(6, NT):
                stage3(t)

            # ---- ship all four column-group partials (no on-device add) ----
            # copy halves split across DVE and ACT so they run in parallel
            outs = sb.tile([NG * B, OUT], F16, tag="outs")
            nc.vector.tensor_copy(outs[:, 0:OUT // 2], pout[0:NG * B, 0:OUT // 2])
            nc.scalar.copy(outs[:, OUT // 2:OUT], pout[0:NG * B, OUT // 2:OUT])
            nc.sync.dma_start(out=out.ap(), in_=outs[:, :])

    # Strip the second exit butterfly barrier: the exit block ends with
    # [tile-drain(SP, sem waits), butterfly A, InstISA sem-range-clear,
    # butterfly B].  Butterfly A + the clear already quiesce every engine and
    # reset the tile sems for the next execution of this NEFF; butterfly B
    # only delays the runtime epilogue.
    exit_blk = nc.main_func.blocks[-1]
    insts = exit_blk.instructions
    isa_idx = max(i for i, ins in enumerate(insts) if isinstance(ins, mybir.InstISA))
    tail = insts[isa_idx + 1:]
    assert all(isinstance(i, (mybir.InstDrain, mybir.InstEventSemaphore)) for i in tail), \
        [type(i).__name__ for i in tail]
    del insts[isa_idx + 1:]

    nc.compile()
    return nc


_NC_CACHE = None


def _get_nc():
    global _NC_CACHE
    if _NC_CACHE is None:
        _NC_CACHE = _build_bass()
    return _NC_CACHE


_CST = _consts()


def make_in_maps(imgs, weight):
    """Per-core input dicts: shuffled channel-0 row slice + weight shard."""
    wr = weight.reshape(OUT, H // BS, WD // BS, NF)  # [o, i_glob, j, m]
    in_maps = []
    for c in range(N_CORES):
        xc = imgs[:, 0, RPC * c:RPC * (c + 1), :]    # [32, 64, 512]
        # [b, (i,a), (wgp, wg2, j16, l)] -> [wgp, (j16, l), (wg2, a, i, b)]
        xd = xc.reshape(B, IPC, BS, 2, 2, 128).transpose(3, 5, 4, 2, 1, 0)
        xd = np.ascontiguousarray(xd.reshape(2, 128, 2 * XCK).astype(np.float16))
        wc = wr[:, IPC * c:IPC * (c + 1)]            # [o, i, j, m]
        # q = wg2*64 + j16*4 + m  (j = (2p + wg2)*16 + j16),  t = p*8 + i
        wtc = wc.reshape(OUT, IPC, 2, 2, 16, NF)     # o, i, p, wg2, j16, m
        wtc = wtc.transpose(3, 4, 5, 2, 1, 0)        # wg2, j16, m, p, i, o
        wtc = (wtc.reshape(128, NT * OUT) * WSCALE).astype(ml_dtypes.float8_e3m4)
        in_maps.append({
            "x": xd,
            "gb": np.ascontiguousarray(_CST),
            "wt": np.ascontiguousarray(wtc),
        })
    return in_maps


def kernel(imgs_tensors, weight, bias, block_size=8, num_features=4, **_):
    assert int(block_size) == BS and int(num_features) == NF
    imgs = np.ascontiguousarray(np.asarray(imgs_tensors, dtype=np.float32))
    w = np.ascontiguousarray(np.asarray(weight, dtype=np.float32))
    b = np.asarray(bias, dtype=np.float32)
    assert imgs.shape == (B, 3, H, WD) and w.shape == (OUT, H // BS * WD // BS * NF)

    nc = _get_nc()
    res = run_bass_kernel_spmd(nc, make_in_maps(imgs, w), core_ids=list(range(N_CORES)))
    acc = np.zeros((B, OUT), np.float32)
    for r in res.results:
        po = r["out"].astype(np.float32)
        for g in range(NG):
            acc += po[g * B:(g + 1) * B]
    return (acc / WSCALE + b[None, :]).astype(np.float32)
